# revision 2
# baseline (speedup 1.0000x reference)
"""Trainium2 Bass kernel (v16) for batched tiny-graph GNN (B=32768, N=22).

Numerics: fp32r matmuls (~12-bit mantissa, bf16-class speed) with exactness
restored where it matters:
  * X shipped as interleaved bf16-exact split (xh, xl); MP1 contracts the
    pair in ONE matmul via K-stacking (lhsT [128, 32]).
  * W1 shipped row-duplicated (w1h2/w1l2 [32, 128]) to consume the
    interleaved AX pair directly; 2 matmuls restore W1 exactly.
  * W2g shipped column-interleaved (w2g2 [128, 128]); hi/lo halves summed by
    one strided DVE add after the Zg matmul.
  * W2s single fp32r; final MLP true fp32 batched per block; A fp32r
    (host-transposed, no PE transposes).

Per wave (16 graphs): MP1 4x88c, d1 2x352c, Zs 1x352c, Zg 4x128c, MP2 4x88c.
Block-diagonal A^T built once per 128-graph block by 4 large scatter copies.
"""

import sys

sys.path.insert(0, "/opt/trn_rl_repo")

import numpy as np
import ml_dtypes

import concourse.bass as bass
import concourse.mybir as mybir
import concourse.tile as tile
from concourse.bass_utils import run_bass_kernel_spmd

import bass_rust


def _patched_drain_and_barrier(self, tick_clock, wait_clock):
    """Walrus in this container cannot encode multi-wait Drain instructions;
    spread the tile-exit sem waits across single-wait NOPs instead."""
    from concourse.tile import ScopedClock

    probe = self.nc.sync.nop(hint="drain_wait_split")
    wait_clock.add_sem_waits(probe.ins, ScopedClock({None: tick_clock.global_clock}))
    si = probe.ins.sync_info
    waits = list(si.on_wait) if si is not None else []
    probe.ins.sync_info = bass_rust.SyncInfo(on_wait=waits[:1], on_update=[])
    for w in waits[1:]:
        n = self.nc.sync.nop(hint="drain_wait_split")
        n.ins.sync_info = bass_rust.SyncInfo(on_wait=[w], on_update=[])
    self.nc.sync.drain()
    self.nc.all_engine_barrier()
    assert self.sems is not None
    popped = self.nc._tile_sem_poison_stack.pop()
    assert popped is self._sem_poison
    self.nc.clear_and_free_semaphores(list(self.sems.allocated().values()))
    self.nc.all_engine_barrier()


tile.TileContext._drain_and_barrier = _patched_drain_and_barrier

F32 = mybir.dt.float32
F32R = mybir.dt.float32r
F16 = mybir.dt.float16

B_TOTAL = 32768
N_CORES = 8
B_CORE = B_TOTAL // N_CORES          # 4096
NN = 22
FIN = 16
C1, C2, C3 = 128, 64, 32
WAVE = 16                            # graphs per wave (4 tetrads)
BLK = 128                            # graphs per DMA ingest block
AFT = mybir.ActivationFunctionType

_split_ctr = [0]


def _split_multi_waits(nc):
    """This container's walrus encodes at most one sem wait per instruction:
    hoist extra waits onto same-engine NOPs inserted just before."""
    for f in nc.m.functions:
        for bb in f.blocks:
            out = []
            for inst in bb.instructions:
                si = inst.sync_info
                if si is not None and len(si.on_wait) > 1:
                    waits = list(si.on_wait)
                    for w in waits[:-1]:
                        _split_ctr[0] += 1
                        n = mybir.InstNoOp(
                            name=f"waitsplit_{_split_ctr[0]}", ins=[], outs=[]
                        )
                        n.engine = inst.engine
                        n.sync_info = bass_rust.SyncInfo(on_wait=[w], on_update=[])
                        out.append(n)
                        nc.register_instruction(n, overwrite=True)
                    inst.sync_info = bass_rust.SyncInfo(
                        on_wait=[waits[-1]], on_update=list(si.on_update)
                    )
                out.append(inst)
            bb.instructions = out


def build_nc(b_core: int = B_CORE, blk: int = BLK) -> bass.Bass:
    assert b_core % blk == 0
    n_blocks = b_core // blk
    waves_per_blk = blk // WAVE

    nc = bass.Bass()

    at_d = nc.declare_dram_parameter("at", [b_core, NN, NN], F32R, isOutput=False)
    x2_d = nc.declare_dram_parameter("x2", [b_core, NN, 2 * FIN], F32R, isOutput=False)
    w1h_d = nc.declare_dram_parameter("w1h2", [2 * FIN, C1], F32R, isOutput=False)
    w1l_d = nc.declare_dram_parameter("w1l2", [2 * FIN, C1], F32R, isOutput=False)
    w2gh_d = nc.declare_dram_parameter("w2gh", [C1, C2], F16, isOutput=False)
    w2gl_d = nc.declare_dram_parameter("w2gl", [C1, C2], F16, isOutput=False)
    w2s_d = nc.declare_dram_parameter("w2s", [C1, C2], F16, isOutput=False)
    b1_d = nc.declare_dram_parameter("b1", [C1], F32, isOutput=False)
    b2_d = nc.declare_dram_parameter("b2", [C2], F32, isOutput=False)
    wf1_d = nc.declare_dram_parameter("wf1", [C2, C3], F32, isOutput=False)
    bf1_d = nc.declare_dram_parameter("bf1", [C3], F32, isOutput=False)
    wf2_d = nc.declare_dram_parameter("wf2", [C3, 1], F32, isOutput=False)
    bf2_d = nc.declare_dram_parameter("bf2", [1], F32, isOutput=False)
    y_d = nc.declare_dram_parameter("y", [b_core, 1], F32, isOutput=True)

    CMP = 4 * NN          # 88 compact cols per tetrad
    CW = 4 * CMP          # 352 compact cols per wave

    with tile.TileContext(nc) as tc:
        with (
            tc.tile_pool(name="const", bufs=1) as cpool,
            tc.tile_pool(name="ioa", bufs=3) as iopool,
            tc.tile_pool(name="iox", bufs=3) as ioxpool,
            tc.tile_pool(name="bdp", bufs=3) as bdpool,
            tc.tile_pool(name="gio", bufs=2) as gpool,
            tc.tile_pool(name="work", bufs=3) as wpool,
            tc.tile_pool(name="pA", bufs=2, space="PSUM") as pApool,
            tc.tile_pool(name="pH", bufs=2, space="PSUM") as pHpool,
            tc.tile_pool(name="pZ", bufs=2, space="PSUM") as pZpool,
            tc.tile_pool(name="pZg", bufs=1, space="PSUM") as pZgpool,
            tc.tile_pool(name="pG", bufs=1, space="PSUM") as pGpool,
        ):
            # ---- persistent constants (all f32; bitcast to f32r at use) ----
            def wload(dram, p, q, tag, dt=F32):
                t = cpool.tile([p, q], dt, tag=tag)
                nc.sync.dma_start(out=t[:, :], in_=dram[:, :])
                return t

            w1h_sb = wload(w1h_d, 2 * FIN, C1, "w1h2", F32R)
            w1l_sb = wload(w1l_d, 2 * FIN, C1, "w1l2", F32R)
            w2gh_sb = wload(w2gh_d, C1, C2, "w2gh", F16)
            w2gl_sb = wload(w2gl_d, C1, C2, "w2gl", F16)
            w2s_sb = wload(w2s_d, C1, C2, "w2s", F16)
            wf1_sb = wload(wf1_d, C2, C3, "wf1")
            wf2t = cpool.tile([C3, 1], F32, tag="wf2")
            nc.sync.dma_start(out=wf2t[:, :], in_=wf2_d[:, :])

            def bload(dram, p, tag):
                t = cpool.tile([p, 1], F32, tag=tag)
                nc.sync.dma_start(out=t[:, :], in_=dram.rearrange("(c o) -> c o", o=1))
                return t

            b1c = bload(b1_d, C1, "b1")
            b2c = bload(b2_d, C2, "b2")
            bf1c = bload(bf1_d, C3, "bf1")
            bf2c = bload(bf2_d, 1, "bf2")
            y_acc = cpool.tile([1, b_core], F32, tag="yacc")

            # padded H1^T tiles (gap cols harmless: bd gap rows are zero)
            h1s = []
            for par in range(2):
                h1 = cpool.tile([C1, 4 * 128], F16, tag=f"h1p{par}")
                nc.vector.memset(h1[:, :], 0.0)
                h1s.append(h1)

            total_waves = n_blocks * waves_per_blk
            prev = None
            x2v = bd_v = None
            sG_blk = None
            b0 = 0

            def drain_mp2(pv):
                for t in range(4):
                    nc.tensor.matmul(
                        pv["pZZ"][:, CMP * t : CMP * (t + 1)],
                        lhsT=pv["sZg"][:, C2 * t : C2 * (t + 1)],
                        rhs=pv["bd_v"][:, 4 * pv["w"] + t, :, 0:NN],
                        start=False, stop=(t == 3),
                        skip_group_check=True,
                    )

            def drain(pv):
                # previous wave: H2 + pool (+ block finals)
                sH2T = wpool.tile([C2, CW], F32, tag="sH2T")
                nc.scalar.activation(
                    out=sH2T[:, :], in_=pv["pZZ"][:, :], func=AFT.Relu,
                    bias=b2c[:, :],
                )
                nc.vector.reduce_sum(
                    out=pv["sG_blk"][:, WAVE * pv["w"] : WAVE * (pv["w"] + 1)]
                    .rearrange("p (t g) -> p t g", t=4),
                    in_=sH2T.rearrange("p (t g q) -> p t g q", t=4, g=4),
                    axis=mybir.AxisListType.X,
                )
                if pv["w"] == waves_per_blk - 1:
                    pGb = pGpool.tile([C3, 2 * blk], F32, tag="pGb")
                    pG1 = pGb[:, 0:blk]
                    nc.tensor.matmul(pG1, lhsT=wf1_sb[:, :], rhs=pv["sG_blk"][:, :])
                    sG1 = gpool.tile([C3, blk], F32, tag="sG1")
                    nc.scalar.activation(
                        out=sG1[:, :], in_=pG1, func=AFT.Relu, bias=bf1c[:, :]
                    )
                    pY = pGb[0:1, blk : 2 * blk]
                    nc.tensor.matmul(pY, lhsT=wf2t[:, :], rhs=sG1[:, :])
                    nc.scalar.activation(
                        out=y_acc[:, pv["b0"] : pv["b0"] + blk], in_=pY,
                        func=AFT.Sigmoid, bias=bf2c[:, :],
                    )

            for wv in range(total_waves):
                iblk, w = wv // waves_per_blk, wv % waves_per_blk
                if w == 0:
                    b0 = iblk * blk
                    # ---- block ingest ----
                    at_blk = iopool.tile([128, (blk // 4) * NN], F32R, tag="at_blk")
                    at_src = at_d[b0 : b0 + blk].rearrange(
                        "(s g) m n -> g m s n", g=4
                    )
                    for g in range(4):
                        nc.sync.dma_start(
                            out=at_blk[32 * g : 32 * g + NN].rearrange(
                                "p (s n) -> p s n", n=NN
                            ),
                            in_=at_src[g],
                        )
                    x2_blk = ioxpool.tile(
                        [128, (blk // 4) * 2 * FIN], F32R, tag="x2_blk"
                    )
                    x2_src = x2_d[b0 : b0 + blk].rearrange(
                        "(s g) m c -> g m s c", g=4
                    )
                    for g in range(4):
                        nc.sync.dma_start(
                            out=x2_blk[32 * g : 32 * g + NN].rearrange(
                                "p (s c) -> p s c", c=2 * FIN
                            ),
                            in_=x2_src[g],
                        )
                    x2v = x2_blk[:, :].rearrange("p (s c) -> p s c", c=2 * FIN)

                    # ---- block-level BD scatter ----
                    bd_all = bdpool.tile(
                        [128, (blk // 4) * 128], F32R, tag="bd_all"
                    )
                    if iblk < 3:
                        nc.vector.memset(bd_all[:, :].bitcast(F32), 0.0)
                    at_v = at_blk[:, :].rearrange("p (s n) -> p s n", n=NN)
                    for g in range(4):
                        dst = bd_all[32 * g : 32 * g + NN].rearrange(
                            "p (s c) -> p s c", c=128
                        )[:, :, 32 * g : 32 * g + NN]
                        srcv = at_v[32 * g : 32 * g + NN]
                        if g < 2:
                            nc.vector.tensor_copy(dst, srcv)
                        elif g == 2:
                            nc.scalar.copy(out=dst, in_=srcv)
                        else:
                            nc.gpsimd.tensor_copy(dst, srcv)
                    bd_v = bd_all[:, :].rearrange(
                        "p (s g q) -> p s g q", g=4, q=32
                    )
                    sG_blk = gpool.tile([C2, blk], F32, tag="sG_blk")

                sH1T = h1s[wv % 2]

                def bd_c(t):
                    return bd_v[:, 4 * w + t, :, 0:NN]

                # ---- MP1: one K-stacked matmul per tetrad ----
                pAXT = pApool.tile([2 * FIN, CW], F32, tag="pAXT")
                for t in range(4):
                    s = 4 * w + t
                    nc.tensor.matmul(
                        pAXT[:, CMP * t : CMP * (t + 1)],
                        lhsT=x2v[:, s, :],
                        rhs=bd_c(t),
                        start=True, stop=True,
                    )
                sAXT = wpool.tile([2 * FIN, CW], F32R, tag="sAXT")
                nc.vector.tensor_copy(sAXT[:, :], pAXT[:, :])

                # ---- dense1 (w1 pair, K-stacked): H1^T = relu(. + b1) ----
                pH1 = pHpool.tile([C1, CW], F32, tag="pH1")
                nc.tensor.matmul(pH1[:, :], lhsT=w1h_sb[:, :], rhs=sAXT[:, :],
                                 start=True, stop=False)
                nc.tensor.matmul(pH1[:, :], lhsT=w1l_sb[:, :], rhs=sAXT[:, :],
                                 start=False, stop=True,
                                 skip_group_check=True)
                h1_strided = sH1T.rearrange("p (t g q) -> p t g q", t=4, g=4)[
                    :, :, :, 0:NN
                ]
                nc.scalar.activation(
                    out=h1_strided, in_=pH1[:, :], func=AFT.Relu, bias=b1c[:, :]
                )
                sH1r = sH1T[:, :]
                h1_rhs = sH1r.rearrange("p (t g q) -> p t g q", t=4, g=4)[
                    :, :, :, 0:NN
                ]

                # ---- Zs into pZZ; Zg pair (fp16) ----
                pZZ = pZpool.tile([C2, CW], F32, tag="pZZ")
                nc.tensor.matmul(pZZ[:, :], lhsT=w2s_sb[:, :],
                                 rhs=h1_rhs, start=True, stop=False)
                pZg = pZgpool.tile([128, 4 * C2], F32, tag="pZg")
                for t in range(4):
                    nc.tensor.matmul(
                        pZg[:, C2 * t : C2 * (t + 1)],
                        lhsT=sH1r[:, 128 * t : 128 * (t + 1)],
                        rhs=w2gh_sb[:, :],
                        start=True, stop=False,
                    )
                    nc.tensor.matmul(
                        pZg[:, C2 * t : C2 * (t + 1)],
                        lhsT=sH1r[:, 128 * t : 128 * (t + 1)],
                        rhs=w2gl_sb[:, :],
                        start=False, stop=True,
                        skip_group_check=True,
                    )
                sZg = wpool.tile([128, 4 * C2], F32R, tag="sZg")
                nc.vector.tensor_copy(sZg[:, :], pZg[:, :])

                # ---- drain previous wave (its sZg has long settled) ----
                if prev is not None:
                    drain_mp2(prev)
                    drain(prev)

                prev = {"pZZ": pZZ, "sZg": sZg, "bd_v": bd_v, "w": w,
                        "sG_blk": sG_blk, "b0": b0}

            drain_mp2(prev)
            drain(prev)

            nc.sync.dma_start(
                out=y_d.rearrange("(o b) one -> o (b one)", o=1), in_=y_acc[:, :]
            )

    _split_multi_waits(nc)
    return nc


def _prep_host(x, a, weights):
    """Host-side layout/precision prep (pure repacking, no graph math)."""
    at = np.ascontiguousarray(a.transpose(0, 2, 1))
    xh = x.astype(ml_dtypes.bfloat16).astype(np.float32)
    xl = (x - xh).astype(np.float32)
    x2 = np.empty(x.shape[:-1] + (2 * FIN,), np.float32)
    x2[..., 0::2] = xh
    x2[..., 1::2] = xl
    w1 = weights["w1"]
    w1h = w1.astype(ml_dtypes.bfloat16).astype(np.float32)
    w1l = (w1 - w1h).astype(np.float32)
    w2g = weights["w2g"]
    w2gh = w2g.astype(np.float16)
    w2gl = (w2g - w2gh.astype(np.float32)).astype(np.float16)
    return {
        "at": at, "x2": x2,
        "w1h2": np.repeat(w1h, 2, axis=0),
        "w1l2": np.repeat(w1l, 2, axis=0),
        "w2gh": w2gh, "w2gl": w2gl,
        "w2s": weights["w2s"].astype(np.float16),
        "b1": weights["b1"], "b2": weights["b2"],
        "wf1": weights["wf1"], "bf1": weights["bf1"],
        "wf2": weights["wf2"], "bf2": weights["bf2"],
    }


def kernel(**inputs) -> np.ndarray:
    x = np.asarray(inputs["x"], dtype=np.float32)
    a = np.asarray(inputs["a"], dtype=np.float32)
    weights = {
        k: np.asarray(inputs[k], dtype=np.float32)
        for k in ("w1", "b1", "w2g", "w2s", "b2", "wf1", "bf1", "wf2", "bf2")
    }
    full = _prep_host(x, a, weights)

    nc = build_nc(B_CORE)
    in_maps = []
    for c in range(N_CORES):
        sl = slice(c * B_CORE, (c + 1) * B_CORE)
        m = {k: (v[sl] if k in ("at", "x2") else v) for k, v in full.items()}
        in_maps.append(m)

    res = run_bass_kernel_spmd(nc, in_maps, list(range(N_CORES)))
    outs = [res.results[c]["y"] for c in range(N_CORES)]
    return np.concatenate(outs, axis=0).astype(np.float32)


if __name__ == "__main__":
    # quick 1-core correctness check against numpy on a small slice
    rng = np.random.default_rng(0)
    bc = 512
    x = rng.standard_normal((bc, NN, FIN), dtype=np.float32)
    a = rng.random((bc, NN, NN), dtype=np.float32)
    w = {
        "w1": rng.standard_normal((FIN, C1), dtype=np.float32) * 0.2,
        "b1": rng.standard_normal(C1).astype(np.float32) * 0.1,
        "w2g": rng.standard_normal((C1, C2), dtype=np.float32) * 0.1,
        "w2s": rng.standard_normal((C1, C2), dtype=np.float32) * 0.1,
        "b2": rng.standard_normal(C2).astype(np.float32) * 0.1,
        "wf1": rng.standard_normal((C2, C3), dtype=np.float32) * 0.1,
        "bf1": rng.standard_normal(C3).astype(np.float32) * 0.1,
        "wf2": rng.standard_normal((C3, 1), dtype=np.float32) * 0.3,
        "bf2": rng.standard_normal(1).astype(np.float32) * 0.1,
    }
    full = _prep_host(x, a, w)
    nc = build_nc(bc)
    res = run_bass_kernel_spmd(nc, [full], [0])
    y = res.results[0]["y"][:, 0]

    AX = a @ x
    H1 = np.maximum(AX @ w["w1"] + w["b1"], 0)
    H2 = np.maximum(a @ (H1 @ w["w2g"]) + H1 @ w["w2s"] + w["b2"], 0)
    g = H2.sum(axis=1)
    g1 = np.maximum(g @ w["wf1"] + w["bf1"], 0)
    z = g1 @ w["wf2"] + w["bf2"]
    yref = (1 / (1 + np.exp(-z)))[:, 0]
    print("max abs err:", np.abs(y - yref).max(), " ref max:", np.abs(yref).max())


# revision 3
# speedup vs baseline: 1.0222x; 1.0222x over previous
"""Trainium2 Bass kernel (final) for batched tiny-graph GNN (B=32768, N=22).

Numerics: fp32r matmuls (~12-bit mantissa, bf16-class speed) with exactness
restored where it matters:
  * X shipped as interleaved bf16-exact split (xh, xl); MP1 contracts the
    pair in ONE matmul via K-stacking (lhsT [128, 32]).
  * W1 shipped row-duplicated (w1h2/w1l2 [32, 128]) to consume the
    interleaved AX pair directly; 2 matmuls restore W1 exactly.
  * W2g shipped column-interleaved (w2g2 [128, 128]); hi/lo halves summed by
    one strided DVE add after the Zg matmul.
  * W2s single fp32r; final MLP true fp32 batched per block; A fp32r
    (host-transposed, no PE transposes).

Per wave (16 graphs): MP1 4x88c, d1 2x352c, Zs 1x352c, Zg 4x128c, MP2 4x88c.
Block-diagonal A^T built once per 128-graph block by 4 large scatter copies.
"""

import sys

sys.path.insert(0, "/opt/trn_rl_repo")

import numpy as np
import ml_dtypes

import concourse.bass as bass
import concourse.mybir as mybir
import concourse.tile as tile
from concourse.bass_utils import run_bass_kernel_spmd

import bass_rust


def _patched_drain_and_barrier(self, tick_clock, wait_clock):
    """Walrus in this container cannot encode multi-wait Drain instructions;
    spread the tile-exit sem waits across single-wait NOPs instead."""
    from concourse.tile import ScopedClock

    probe = self.nc.sync.nop(hint="drain_wait_split")
    wait_clock.add_sem_waits(probe.ins, ScopedClock({None: tick_clock.global_clock}))
    si = probe.ins.sync_info
    waits = list(si.on_wait) if si is not None else []
    probe.ins.sync_info = bass_rust.SyncInfo(on_wait=waits[:1], on_update=[])
    for w in waits[1:]:
        n = self.nc.sync.nop(hint="drain_wait_split")
        n.ins.sync_info = bass_rust.SyncInfo(on_wait=[w], on_update=[])
    self.nc.sync.drain()
    self.nc.all_engine_barrier()
    assert self.sems is not None
    popped = self.nc._tile_sem_poison_stack.pop()
    assert popped is self._sem_poison
    self.nc.clear_and_free_semaphores(list(self.sems.allocated().values()))
    self.nc.all_engine_barrier()


tile.TileContext._drain_and_barrier = _patched_drain_and_barrier

F32 = mybir.dt.float32
F32R = mybir.dt.float32r
F16 = mybir.dt.float16

B_TOTAL = 32768
N_CORES = 8
B_CORE = B_TOTAL // N_CORES          # 4096
NN = 22
FIN = 16
C1, C2, C3 = 128, 64, 32
WAVE = 16                            # graphs per wave (4 tetrads)
BLK = 128                            # graphs per DMA ingest block
AFT = mybir.ActivationFunctionType

_split_ctr = [0]


def _split_multi_waits(nc):
    """This container's walrus encodes at most one sem wait per instruction:
    hoist extra waits onto same-engine NOPs inserted just before."""
    for f in nc.m.functions:
        for bb in f.blocks:
            out = []
            for inst in bb.instructions:
                si = inst.sync_info
                if si is not None and len(si.on_wait) > 1:
                    waits = list(si.on_wait)
                    for w in waits[:-1]:
                        _split_ctr[0] += 1
                        n = mybir.InstNoOp(
                            name=f"waitsplit_{_split_ctr[0]}", ins=[], outs=[]
                        )
                        n.engine = inst.engine
                        n.sync_info = bass_rust.SyncInfo(on_wait=[w], on_update=[])
                        out.append(n)
                        nc.register_instruction(n, overwrite=True)
                    inst.sync_info = bass_rust.SyncInfo(
                        on_wait=[waits[-1]], on_update=list(si.on_update)
                    )
                out.append(inst)
            bb.instructions = out


def build_nc(b_core: int = B_CORE, blk: int = BLK) -> bass.Bass:
    assert b_core % blk == 0
    n_blocks = b_core // blk
    waves_per_blk = blk // WAVE

    nc = bass.Bass()

    at_d = nc.declare_dram_parameter("at", [b_core, NN, NN], F32R, isOutput=False)
    x2_d = nc.declare_dram_parameter("x2", [b_core, NN, 2 * FIN], F32R, isOutput=False)
    w1h_d = nc.declare_dram_parameter("w1h2", [2 * FIN, C1], F32R, isOutput=False)
    w1l_d = nc.declare_dram_parameter("w1l2", [2 * FIN, C1], F32R, isOutput=False)
    w2gh_d = nc.declare_dram_parameter("w2gh", [C1, C2], F16, isOutput=False)
    w2gl_d = nc.declare_dram_parameter("w2gl", [C1, C2], F16, isOutput=False)
    w2s_d = nc.declare_dram_parameter("w2s", [C1, C2], F16, isOutput=False)
    b1_d = nc.declare_dram_parameter("b1", [C1], F32, isOutput=False)
    b2_d = nc.declare_dram_parameter("b2", [C2], F32, isOutput=False)
    wf1_d = nc.declare_dram_parameter("wf1", [C2, C3], F32, isOutput=False)
    bf1_d = nc.declare_dram_parameter("bf1", [C3], F32, isOutput=False)
    wf2_d = nc.declare_dram_parameter("wf2", [C3, 1], F32, isOutput=False)
    bf2_d = nc.declare_dram_parameter("bf2", [1], F32, isOutput=False)
    y_d = nc.declare_dram_parameter("y", [b_core, 1], F32, isOutput=True)

    CMP = 4 * NN          # 88 compact cols per tetrad
    CW = 4 * CMP          # 352 compact cols per wave

    with tile.TileContext(nc) as tc:
        with (
            tc.tile_pool(name="const", bufs=1) as cpool,
            tc.tile_pool(name="ioa", bufs=3) as iopool,
            tc.tile_pool(name="iox", bufs=3) as ioxpool,
            tc.tile_pool(name="bdp", bufs=3) as bdpool,
            tc.tile_pool(name="gio", bufs=2) as gpool,
            tc.tile_pool(name="work", bufs=2) as wpool,
            tc.tile_pool(name="pA", bufs=2, space="PSUM") as pApool,
            tc.tile_pool(name="pH", bufs=2, space="PSUM") as pHpool,
            tc.tile_pool(name="pZ", bufs=2, space="PSUM") as pZpool,
            tc.tile_pool(name="pZg", bufs=1, space="PSUM") as pZgpool,
        ):
            # ---- persistent constants (all f32; bitcast to f32r at use) ----
            def wload(dram, p, q, tag, dt=F32):
                t = cpool.tile([p, q], dt, tag=tag)
                nc.sync.dma_start(out=t[:, :], in_=dram[:, :])
                return t

            w1h_sb = wload(w1h_d, 2 * FIN, C1, "w1h2", F32R)
            w1l_sb = wload(w1l_d, 2 * FIN, C1, "w1l2", F32R)
            w2gh_sb = wload(w2gh_d, C1, C2, "w2gh", F16)
            w2gl_sb = wload(w2gl_d, C1, C2, "w2gl", F16)
            w2s_sb = wload(w2s_d, C1, C2, "w2s", F16)
            wf1_sb = wload(wf1_d, C2, C3, "wf1")
            wf2t = cpool.tile([C3, 1], F32, tag="wf2")
            nc.sync.dma_start(out=wf2t[:, :], in_=wf2_d[:, :])

            def bload(dram, p, tag):
                t = cpool.tile([p, 1], F32, tag=tag)
                nc.sync.dma_start(out=t[:, :], in_=dram.rearrange("(c o) -> c o", o=1))
                return t

            b1c = bload(b1_d, C1, "b1")
            b2c = bload(b2_d, C2, "b2")
            bf1c = bload(bf1_d, C3, "bf1")
            bf2c = bload(bf2_d, 1, "bf2")
            y_acc = cpool.tile([1, b_core], F32, tag="yacc")

            # padded H1^T tiles (gap cols harmless: bd gap rows are zero)
            h1s = []
            for par in range(2):
                h1 = cpool.tile([C1, 4 * 128], F16, tag=f"h1p{par}")
                nc.vector.memset(h1[:, :], 0.0)
                h1s.append(h1)

            total_waves = n_blocks * waves_per_blk
            prev = None
            x2v = bd_v = None
            sG_blk = None
            b0 = 0

            def drain_mp2(pv):
                for t in range(4):
                    nc.tensor.matmul(
                        pv["pZZ"][:, CMP * t : CMP * (t + 1)],
                        lhsT=pv["sZg"][:, C2 * t : C2 * (t + 1)],
                        rhs=pv["bd_v"][:, 4 * pv["w"] + t, :, 0:NN],
                        start=False, stop=(t == 3),
                        skip_group_check=True,
                    )

            def drain(pv):
                # previous wave: H2 + pool (+ block finals)
                sH2T = wpool.tile([C2, CW], F32, tag="sH2T")
                nc.scalar.activation(
                    out=sH2T[:, :], in_=pv["pZZ"][:, :], func=AFT.Relu,
                    bias=b2c[:, :],
                )
                nc.vector.reduce_sum(
                    out=pv["sG_blk"][:, WAVE * pv["w"] : WAVE * (pv["w"] + 1)]
                    .rearrange("p (t g) -> p t g", t=4),
                    in_=sH2T.rearrange("p (t g q) -> p t g q", t=4, g=4),
                    axis=mybir.AxisListType.X,
                )
                if pv["w"] == waves_per_blk - 1:
                    pGb = pZgpool.tile([128, 4 * C2], F32, tag="pZg")
                    pG1 = pGb[0:C3, 0:blk]
                    nc.tensor.matmul(pG1, lhsT=wf1_sb[:, :], rhs=pv["sG_blk"][:, :])
                    sG1 = gpool.tile([C3, blk], F32, tag="sG1")
                    nc.scalar.activation(
                        out=sG1[:, :], in_=pG1, func=AFT.Relu, bias=bf1c[:, :]
                    )
                    pY = pGb[0:1, blk : 2 * blk]
                    nc.tensor.matmul(pY, lhsT=wf2t[:, :], rhs=sG1[:, :])
                    nc.scalar.activation(
                        out=y_acc[:, pv["b0"] : pv["b0"] + blk], in_=pY,
                        func=AFT.Sigmoid, bias=bf2c[:, :],
                    )

            prim = cpool.tile([128, 128], F16, tag="prim")
            nc.vector.memset(prim[:, :], 0.0)
            scratch = nc.alloc_psum_tensor("ham_scratch", [128, 128], F32)

            def filler(n):
                for _ in range(n):
                    nc.tensor.matmul(scratch[:, :], lhsT=prim[:, :],
                                     rhs=prim[:, :], start=True, stop=True,
                                     skip_group_check=True)

            filler(72)

            def ingest(kblk):
                # issue the DMAs + allocate tiles for block kblk
                kb0 = kblk * blk
                at_blk = iopool.tile([128, (blk // 4) * NN], F32R, tag="at_blk")
                at_src = at_d[kb0 : kb0 + blk].rearrange(
                    "(s g) m n -> g m s n", g=4
                )
                for g in range(4):
                    nc.sync.dma_start(
                        out=at_blk[32 * g : 32 * g + NN].rearrange(
                            "p (s n) -> p s n", n=NN
                        ),
                        in_=at_src[g],
                    )
                x2_blk = ioxpool.tile(
                    [128, (blk // 4) * 2 * FIN], F32R, tag="x2_blk"
                )
                x2_src = x2_d[kb0 : kb0 + blk].rearrange(
                    "(s g) m c -> g m s c", g=4
                )
                for g in range(4):
                    nc.sync.dma_start(
                        out=x2_blk[32 * g : 32 * g + NN].rearrange(
                            "p (s c) -> p s c", c=2 * FIN
                        ),
                        in_=x2_src[g],
                    )
                bd_all = bdpool.tile([128, (blk // 4) * 128], F32R, tag="bd_all")
                if kblk < 3:
                    nc.vector.memset(bd_all[:, :].bitcast(F32), 0.0)
                return {"at_blk": at_blk, "x2_blk": x2_blk, "bd_all": bd_all}

            def scatter_slice(blkh, j):
                # one-eighth of the BD scatter for a block, on gpsimd
                g, half = j // 2, j % 2
                sw = (blk // 8)
                at_v = blkh["at_blk"][:, :].rearrange("p (s n) -> p s n", n=NN)
                dst = blkh["bd_all"][32 * g : 32 * g + NN].rearrange(
                    "p (s c) -> p s c", c=128
                )[:, half * sw : (half + 1) * sw, 32 * g : 32 * g + NN]
                srcv = at_v[32 * g : 32 * g + NN, half * sw : (half + 1) * sw]
                nc.gpsimd.tensor_copy(dst, srcv)

            cur = ingest(0)
            for j in range(8):
                scatter_slice(cur, j)
            nxt = ingest(1) if n_blocks > 1 else None

            for wv in range(total_waves):
                iblk, w = wv // waves_per_blk, wv % waves_per_blk
                if w == 0:
                    b0 = iblk * blk
                    x2v = cur["x2_blk"][:, :].rearrange(
                        "p (s c) -> p s c", c=2 * FIN
                    )
                    bd_v = cur["bd_all"][:, :].rearrange(
                        "p (s g q) -> p s g q", g=4, q=32
                    )
                    sG_blk = gpool.tile([C2, blk], F32, tag="sG_blk")
                    if iblk > 0:
                        filler(18)
                # prefetch next block: scatter one slice per wave on gpsimd
                if nxt is not None:
                    scatter_slice(nxt, w)
                    if w == waves_per_blk - 1:
                        cur = nxt
                        nxt = ingest(iblk + 2) if iblk + 2 < n_blocks else None

                sH1T = h1s[wv % 2]

                def bd_c(t):
                    return bd_v[:, 4 * w + t, :, 0:NN]

                # ---- MP1: one K-stacked matmul per tetrad ----
                pAXT = pApool.tile([2 * FIN, CW], F32, tag="pAXT")
                for t in range(4):
                    s = 4 * w + t
                    nc.tensor.matmul(
                        pAXT[:, CMP * t : CMP * (t + 1)],
                        lhsT=x2v[:, s, :],
                        rhs=bd_c(t),
                        start=True, stop=True,
                    )
                sAXT = wpool.tile([2 * FIN, CW], F32R, tag="sAXT")
                nc.vector.tensor_copy(sAXT[:, :], pAXT[:, :])

                # ---- dense1 (w1 pair, K-stacked): H1^T = relu(. + b1) ----
                pH1 = pHpool.tile([C1, CW], F32, tag="pH1")
                nc.tensor.matmul(pH1[:, :], lhsT=w1h_sb[:, :], rhs=sAXT[:, :],
                                 start=True, stop=False)
                nc.tensor.matmul(pH1[:, :], lhsT=w1l_sb[:, :], rhs=sAXT[:, :],
                                 start=False, stop=True,
                                 skip_group_check=True)
                h1_strided = sH1T.rearrange("p (t g q) -> p t g q", t=4, g=4)[
                    :, :, :, 0:NN
                ]
                nc.scalar.activation(
                    out=h1_strided, in_=pH1[:, :], func=AFT.Relu, bias=b1c[:, :]
                )
                sH1r = sH1T[:, :]
                h1_rhs = sH1r.rearrange("p (t g q) -> p t g q", t=4, g=4)[
                    :, :, :, 0:NN
                ]

                # ---- Zs into pZZ; Zg pair (fp16) ----
                pZZ = pZpool.tile([C2, CW], F32, tag="pZZ")
                nc.tensor.matmul(pZZ[:, :], lhsT=w2s_sb[:, :],
                                 rhs=h1_rhs, start=True, stop=False)
                pZg = pZgpool.tile([128, 4 * C2], F32, tag="pZg")
                for t in range(4):
                    nc.tensor.matmul(
                        pZg[:, C2 * t : C2 * (t + 1)],
                        lhsT=sH1r[:, 128 * t : 128 * (t + 1)],
                        rhs=w2gh_sb[:, :],
                        start=True, stop=False,
                    )
                    nc.tensor.matmul(
                        pZg[:, C2 * t : C2 * (t + 1)],
                        lhsT=sH1r[:, 128 * t : 128 * (t + 1)],
                        rhs=w2gl_sb[:, :],
                        start=False, stop=True,
                        skip_group_check=True,
                    )
                sZg = wpool.tile([128, 4 * C2], F32R, tag="sZg")
                nc.vector.tensor_copy(sZg[:, :], pZg[:, :])

                # ---- drain previous wave (its sZg has long settled) ----
                if prev is not None:
                    drain_mp2(prev)
                    drain(prev)

                prev = {"pZZ": pZZ, "sZg": sZg, "bd_v": bd_v, "w": w,
                        "sG_blk": sG_blk, "b0": b0}

            drain_mp2(prev)
            drain(prev)

            nc.sync.dma_start(
                out=y_d.rearrange("(o b) one -> o (b one)", o=1), in_=y_acc[:, :]
            )

    _split_multi_waits(nc)
    return nc


def _prep_host(x, a, weights):
    """Host-side layout/precision prep (pure repacking, no graph math)."""
    at = np.ascontiguousarray(a.transpose(0, 2, 1))
    xh = x.astype(ml_dtypes.bfloat16).astype(np.float32)
    xl = (x - xh).astype(np.float32)
    x2 = np.empty(x.shape[:-1] + (2 * FIN,), np.float32)
    x2[..., 0::2] = xh
    x2[..., 1::2] = xl
    w1 = weights["w1"]
    w1h = w1.astype(ml_dtypes.bfloat16).astype(np.float32)
    w1l = (w1 - w1h).astype(np.float32)
    w2g = weights["w2g"]
    w2gh = w2g.astype(np.float16)
    w2gl = (w2g - w2gh.astype(np.float32)).astype(np.float16)
    return {
        "at": at, "x2": x2,
        "w1h2": np.repeat(w1h, 2, axis=0),
        "w1l2": np.repeat(w1l, 2, axis=0),
        "w2gh": w2gh, "w2gl": w2gl,
        "w2s": weights["w2s"].astype(np.float16),
        "b1": weights["b1"], "b2": weights["b2"],
        "wf1": weights["wf1"], "bf1": weights["bf1"],
        "wf2": weights["wf2"], "bf2": weights["bf2"],
    }


def kernel(**inputs) -> np.ndarray:
    x = np.asarray(inputs["x"], dtype=np.float32)
    a = np.asarray(inputs["a"], dtype=np.float32)
    weights = {
        k: np.asarray(inputs[k], dtype=np.float32)
        for k in ("w1", "b1", "w2g", "w2s", "b2", "wf1", "bf1", "wf2", "bf2")
    }
    full = _prep_host(x, a, weights)

    nc = build_nc(B_CORE)
    in_maps = []
    for c in range(N_CORES):
        sl = slice(c * B_CORE, (c + 1) * B_CORE)
        m = {k: (v[sl] if k in ("at", "x2") else v) for k, v in full.items()}
        in_maps.append(m)

    res = run_bass_kernel_spmd(nc, in_maps, list(range(N_CORES)))
    outs = [res.results[c]["y"] for c in range(N_CORES)]
    return np.concatenate(outs, axis=0).astype(np.float32)


if __name__ == "__main__":
    # quick 1-core correctness check against numpy on a small slice
    rng = np.random.default_rng(0)
    bc = 512
    x = rng.standard_normal((bc, NN, FIN), dtype=np.float32)
    a = rng.random((bc, NN, NN), dtype=np.float32)
    w = {
        "w1": rng.standard_normal((FIN, C1), dtype=np.float32) * 0.2,
        "b1": rng.standard_normal(C1).astype(np.float32) * 0.1,
        "w2g": rng.standard_normal((C1, C2), dtype=np.float32) * 0.1,
        "w2s": rng.standard_normal((C1, C2), dtype=np.float32) * 0.1,
        "b2": rng.standard_normal(C2).astype(np.float32) * 0.1,
        "wf1": rng.standard_normal((C2, C3), dtype=np.float32) * 0.1,
        "bf1": rng.standard_normal(C3).astype(np.float32) * 0.1,
        "wf2": rng.standard_normal((C3, 1), dtype=np.float32) * 0.3,
        "bf2": rng.standard_normal(1).astype(np.float32) * 0.1,
    }
    full = _prep_host(x, a, w)
    nc = build_nc(bc)
    res = run_bass_kernel_spmd(nc, [full], [0])
    y = res.results[0]["y"][:, 0]

    AX = a @ x
    H1 = np.maximum(AX @ w["w1"] + w["b1"], 0)
    H2 = np.maximum(a @ (H1 @ w["w2g"]) + H1 @ w["w2s"] + w["b2"], 0)
    g = H2.sum(axis=1)
    g1 = np.maximum(g @ w["wf1"] + w["bf1"], 0)
    z = g1 @ w["wf2"] + w["bf2"]
    yref = (1 / (1 + np.exp(-z)))[:, 0]
    print("max abs err:", np.abs(y - yref).max(), " ref max:", np.abs(yref).max())


# revision 4
# speedup vs baseline: 1.0246x; 1.0023x over previous
"""Trainium2 Bass kernel v4 for batched tiny-graph GNN (B=32768, N=22).

Numerics: fp32r matmuls (~12-bit mantissa, bf16-class speed) with exactness
restored where it matters:
  * X shipped as interleaved bf16-exact split (xh, xl); MP1 contracts the
    pair in ONE matmul via K-stacking (lhsT [128, 32]).
  * W1 shipped row-duplicated (w1h2/w1l2 [32, 128]) to consume the
    interleaved AX pair directly; 2 matmuls restore W1 exactly.
  * W2g shipped column-interleaved (w2g2 [128, 128]); hi/lo halves summed by
    one strided DVE add after the Zg matmul.
  * W2s single fp32r; final MLP true fp32 batched per block; A fp32r
    (host-transposed, no PE transposes).

Per wave (16 graphs): MP1 4x88c, d1 2x352c, Zs 1x352c, Zg 4x128c, MP2 4x88c.
Block-diagonal A^T built once per 128-graph block by 4 large scatter copies.
"""

import sys

sys.path.insert(0, "/opt/trn_rl_repo")

import numpy as np
import ml_dtypes

import concourse.bass as bass
import concourse.mybir as mybir
import concourse.tile as tile
from concourse.bass_utils import run_bass_kernel_spmd

import bass_rust


def _patched_drain_and_barrier(self, tick_clock, wait_clock):
    """Walrus in this container cannot encode multi-wait Drain instructions;
    spread the tile-exit sem waits across single-wait NOPs instead."""
    from concourse.tile import ScopedClock

    probe = self.nc.sync.nop(hint="drain_wait_split")
    wait_clock.add_sem_waits(probe.ins, ScopedClock({None: tick_clock.global_clock}))
    si = probe.ins.sync_info
    waits = list(si.on_wait) if si is not None else []
    probe.ins.sync_info = bass_rust.SyncInfo(on_wait=waits[:1], on_update=[])
    for w in waits[1:]:
        n = self.nc.sync.nop(hint="drain_wait_split")
        n.ins.sync_info = bass_rust.SyncInfo(on_wait=[w], on_update=[])
    self.nc.sync.drain()
    self.nc.all_engine_barrier()
    assert self.sems is not None
    popped = self.nc._tile_sem_poison_stack.pop()
    assert popped is self._sem_poison
    self.nc.clear_and_free_semaphores(list(self.sems.allocated().values()))
    self.nc.all_engine_barrier()


tile.TileContext._drain_and_barrier = _patched_drain_and_barrier

F32 = mybir.dt.float32
F32R = mybir.dt.float32r
F16 = mybir.dt.float16

B_TOTAL = 32768
N_CORES = 8
B_CORE = B_TOTAL // N_CORES          # 4096
NN = 22
FIN = 16
C1, C2, C3 = 128, 64, 32
WAVE = 16                            # graphs per wave (4 tetrads)
BLK = 256                            # graphs per DMA ingest block
AFT = mybir.ActivationFunctionType

_split_ctr = [0]


def _split_multi_waits(nc):
    """This container's walrus encodes at most one sem wait per instruction:
    hoist extra waits onto same-engine NOPs inserted just before."""
    for f in nc.m.functions:
        for bb in f.blocks:
            out = []
            for inst in bb.instructions:
                si = inst.sync_info
                if si is not None and len(si.on_wait) > 1:
                    waits = list(si.on_wait)
                    for w in waits[:-1]:
                        _split_ctr[0] += 1
                        n = mybir.InstNoOp(
                            name=f"waitsplit_{_split_ctr[0]}", ins=[], outs=[]
                        )
                        n.engine = inst.engine
                        n.sync_info = bass_rust.SyncInfo(on_wait=[w], on_update=[])
                        out.append(n)
                        nc.register_instruction(n, overwrite=True)
                    inst.sync_info = bass_rust.SyncInfo(
                        on_wait=[waits[-1]], on_update=list(si.on_update)
                    )
                out.append(inst)
            bb.instructions = out


def build_nc(b_core: int = B_CORE, blk: int = BLK) -> bass.Bass:
    assert b_core % blk == 0
    n_blocks = b_core // blk
    waves_per_blk = blk // WAVE

    nc = bass.Bass()

    at_d = nc.declare_dram_parameter("at", [b_core, NN, NN], F32R, isOutput=False)
    x2_d = nc.declare_dram_parameter("x2", [b_core, NN, 2 * FIN], F32R, isOutput=False)
    w1h_d = nc.declare_dram_parameter("w1h2", [2 * FIN, C1], F32R, isOutput=False)
    w1l_d = nc.declare_dram_parameter("w1l2", [2 * FIN, C1], F32R, isOutput=False)
    w2gh_d = nc.declare_dram_parameter("w2gh", [C1, C2], F16, isOutput=False)
    w2gl_d = nc.declare_dram_parameter("w2gl", [C1, C2], F16, isOutput=False)
    w2s_d = nc.declare_dram_parameter("w2s", [C1, C2], F16, isOutput=False)
    b1_d = nc.declare_dram_parameter("b1", [C1], F32, isOutput=False)
    b2_d = nc.declare_dram_parameter("b2", [C2], F32, isOutput=False)
    wf1_d = nc.declare_dram_parameter("wf1", [C2, C3], F32, isOutput=False)
    bf1_d = nc.declare_dram_parameter("bf1", [C3], F32, isOutput=False)
    wf2_d = nc.declare_dram_parameter("wf2", [C3, 1], F32, isOutput=False)
    bf2_d = nc.declare_dram_parameter("bf2", [1], F32, isOutput=False)
    y_d = nc.declare_dram_parameter("y", [b_core, 1], F32, isOutput=True)

    CMP = 4 * NN          # 88 compact cols per tetrad
    CW = 4 * CMP          # 352 compact cols per wave

    with tile.TileContext(nc) as tc:
        with (
            tc.tile_pool(name="const", bufs=1) as cpool,
            tc.tile_pool(name="ioa", bufs=2) as iopool,
            tc.tile_pool(name="iox", bufs=2) as ioxpool,
            tc.tile_pool(name="bdp", bufs=2) as bdpool,
            tc.tile_pool(name="gio", bufs=2) as gpool,
            tc.tile_pool(name="work", bufs=2) as wpool,
            tc.tile_pool(name="pA", bufs=2, space="PSUM") as pApool,
            tc.tile_pool(name="pH", bufs=2, space="PSUM") as pHpool,
            tc.tile_pool(name="pZ", bufs=2, space="PSUM") as pZpool,
            tc.tile_pool(name="pZg", bufs=1, space="PSUM") as pZgpool,
        ):
            # ---- persistent constants (all f32; bitcast to f32r at use) ----
            def wload(dram, p, q, tag, dt=F32):
                t = cpool.tile([p, q], dt, tag=tag)
                nc.sync.dma_start(out=t[:, :], in_=dram[:, :])
                return t

            w1h_sb = wload(w1h_d, 2 * FIN, C1, "w1h2", F32R)
            w1l_sb = wload(w1l_d, 2 * FIN, C1, "w1l2", F32R)
            w2gh_sb = wload(w2gh_d, C1, C2, "w2gh", F16)
            w2gl_sb = wload(w2gl_d, C1, C2, "w2gl", F16)
            w2s_sb = wload(w2s_d, C1, C2, "w2s", F16)
            wf1_sb = wload(wf1_d, C2, C3, "wf1")
            wf2t = cpool.tile([C3, 1], F32, tag="wf2")
            nc.sync.dma_start(out=wf2t[:, :], in_=wf2_d[:, :])

            def bload(dram, p, tag):
                t = cpool.tile([p, 1], F32, tag=tag)
                nc.sync.dma_start(out=t[:, :], in_=dram.rearrange("(c o) -> c o", o=1))
                return t

            b1c = bload(b1_d, C1, "b1")
            b2c = bload(b2_d, C2, "b2")
            bf1c = bload(bf1_d, C3, "bf1")
            bf2c = bload(bf2_d, 1, "bf2")
            y_acc = cpool.tile([1, b_core], F32, tag="yacc")

            # padded H1^T tiles (gap cols harmless: bd gap rows are zero)
            h1s = []
            for par in range(2):
                h1 = cpool.tile([C1, 4 * 128], F16, tag=f"h1p{par}")
                nc.vector.memset(h1[:, :], 0.0)
                h1s.append(h1)

            total_waves = n_blocks * waves_per_blk
            prev = None
            x2v = bd_v = None
            sG_blk = None
            b0 = 0

            def drain_mp2(pv):
                for t in range(4):
                    nc.tensor.matmul(
                        pv["pZZ"][:, CMP * t : CMP * (t + 1)],
                        lhsT=pv["sZg"][:, C2 * t : C2 * (t + 1)],
                        rhs=pv["bd_v"][:, 4 * pv["w"] + t, :, 0:NN],
                        start=False, stop=(t == 3),
                        skip_group_check=True,
                    )

            def drain(pv):
                # previous wave: H2 + pool (+ block finals)
                sH2T = wpool.tile([C2, CW], F32, tag="sH2T")
                nc.scalar.activation(
                    out=sH2T[:, :], in_=pv["pZZ"][:, :], func=AFT.Relu,
                    bias=b2c[:, :],
                )
                nc.vector.reduce_sum(
                    out=pv["sG_blk"][:, WAVE * pv["w"] : WAVE * (pv["w"] + 1)]
                    .rearrange("p (t g) -> p t g", t=4),
                    in_=sH2T.rearrange("p (t g q) -> p t g q", t=4, g=4),
                    axis=mybir.AxisListType.X,
                )
                if pv["w"] == waves_per_blk - 1:
                    pGb = pZgpool.tile([128, 4 * C2], F32, tag="pZg")
                    pG1 = pGb[0:C3, 0:blk]
                    nc.tensor.matmul(pG1, lhsT=wf1_sb[:, :], rhs=pv["sG_blk"][:, :])
                    sG1 = gpool.tile([C3, blk], F32, tag="sG1")
                    nc.scalar.activation(
                        out=sG1[:, :], in_=pG1, func=AFT.Relu, bias=bf1c[:, :]
                    )
                    pY = pGb[32:33, 0:blk]
                    nc.tensor.matmul(pY, lhsT=wf2t[:, :], rhs=sG1[:, :])
                    nc.scalar.activation(
                        out=y_acc[:, pv["b0"] : pv["b0"] + blk], in_=pY,
                        func=AFT.Sigmoid, bias=bf2c[:, :],
                    )

            prim = cpool.tile([128, 128], F16, tag="prim")
            nc.vector.memset(prim[:, :], 0.0)
            scratch = nc.alloc_psum_tensor("ham_scratch", [128, 128], F32)

            def filler(n):
                for _ in range(n):
                    nc.tensor.matmul(scratch[:, :], lhsT=prim[:, :],
                                     rhs=prim[:, :], start=True, stop=True,
                                     skip_group_check=True)

            filler(72)

            def ingest(kblk):
                # issue the DMAs + allocate tiles for block kblk
                kb0 = kblk * blk
                at_blk = iopool.tile([128, (blk // 4) * NN], F32R, tag="at_blk")
                at_src = at_d[kb0 : kb0 + blk].rearrange(
                    "(s g) m n -> g m s n", g=4
                )
                for g in range(4):
                    nc.sync.dma_start(
                        out=at_blk[32 * g : 32 * g + NN].rearrange(
                            "p (s n) -> p s n", n=NN
                        ),
                        in_=at_src[g],
                    )
                x2_blk = ioxpool.tile(
                    [128, (blk // 4) * 2 * FIN], F32R, tag="x2_blk"
                )
                x2_src = x2_d[kb0 : kb0 + blk].rearrange(
                    "(s g) m c -> g m s c", g=4
                )
                for g in range(4):
                    nc.sync.dma_start(
                        out=x2_blk[32 * g : 32 * g + NN].rearrange(
                            "p (s c) -> p s c", c=2 * FIN
                        ),
                        in_=x2_src[g],
                    )
                bd_all = bdpool.tile([128, (blk // 4) * 128], F32R, tag="bd_all")
                if kblk < 2:
                    nc.vector.memset(bd_all[:, :].bitcast(F32), 0.0)
                return {"at_blk": at_blk, "x2_blk": x2_blk, "bd_all": bd_all}

            def scatter_slice(blkh, j):
                # one slice (1/waves_per_blk) of the BD scatter, on gpsimd
                g, half = j // (waves_per_blk // 4), j % (waves_per_blk // 4)
                sw = (blk // 4) // (waves_per_blk // 4)
                at_v = blkh["at_blk"][:, :].rearrange("p (s n) -> p s n", n=NN)
                dst = blkh["bd_all"][32 * g : 32 * g + NN].rearrange(
                    "p (s c) -> p s c", c=128
                )[:, half * sw : (half + 1) * sw, 32 * g : 32 * g + NN]
                srcv = at_v[32 * g : 32 * g + NN, half * sw : (half + 1) * sw]
                nc.gpsimd.tensor_copy(dst, srcv)

            cur = ingest(0)
            for j in range(waves_per_blk):
                scatter_slice(cur, j)
            nxt = ingest(1) if n_blocks > 1 else None

            for wv in range(total_waves):
                iblk, w = wv // waves_per_blk, wv % waves_per_blk
                if w == 0:
                    b0 = iblk * blk
                    x2v = cur["x2_blk"][:, :].rearrange(
                        "p (s c) -> p s c", c=2 * FIN
                    )
                    bd_v = cur["bd_all"][:, :].rearrange(
                        "p (s g q) -> p s g q", g=4, q=32
                    )
                    sG_blk = gpool.tile([C2, blk], F32, tag="sG_blk")
                    if iblk > 0:
                        filler(18)
                # prefetch next block: scatter one slice per wave on gpsimd
                if nxt is not None:
                    scatter_slice(nxt, w)
                    if w == waves_per_blk - 1:
                        cur = nxt
                        nxt = ingest(iblk + 2) if iblk + 2 < n_blocks else None

                sH1T = h1s[wv % 2]

                def bd_c(t):
                    return bd_v[:, 4 * w + t, :, 0:NN]

                # ---- MP1: one K-stacked matmul per tetrad ----
                pAXT = pApool.tile([2 * FIN, CW], F32, tag="pAXT")
                for t in range(4):
                    s = 4 * w + t
                    nc.tensor.matmul(
                        pAXT[:, CMP * t : CMP * (t + 1)],
                        lhsT=x2v[:, s, :],
                        rhs=bd_c(t),
                        start=True, stop=True,
                    )
                sAXT = wpool.tile([2 * FIN, CW], F32R, tag="sAXT")
                nc.vector.tensor_copy(sAXT[:, :], pAXT[:, :])

                # ---- dense1 (w1 pair, K-stacked): H1^T = relu(. + b1) ----
                pH1 = pHpool.tile([C1, CW], F32, tag="pH1")
                nc.tensor.matmul(pH1[:, :], lhsT=w1h_sb[:, :], rhs=sAXT[:, :],
                                 start=True, stop=False)
                nc.tensor.matmul(pH1[:, :], lhsT=w1l_sb[:, :], rhs=sAXT[:, :],
                                 start=False, stop=True,
                                 skip_group_check=True)
                h1_strided = sH1T.rearrange("p (t g q) -> p t g q", t=4, g=4)[
                    :, :, :, 0:NN
                ]
                nc.scalar.activation(
                    out=h1_strided, in_=pH1[:, :], func=AFT.Relu, bias=b1c[:, :]
                )
                sH1r = sH1T[:, :]
                h1_rhs = sH1r.rearrange("p (t g q) -> p t g q", t=4, g=4)[
                    :, :, :, 0:NN
                ]

                # ---- Zs into pZZ; Zg pair (fp16) ----
                pZZ = pZpool.tile([C2, CW], F32, tag="pZZ")
                nc.tensor.matmul(pZZ[:, :], lhsT=w2s_sb[:, :],
                                 rhs=h1_rhs, start=True, stop=False)
                pZg = pZgpool.tile([128, 4 * C2], F32, tag="pZg")
                for t in range(4):
                    nc.tensor.matmul(
                        pZg[:, C2 * t : C2 * (t + 1)],
                        lhsT=sH1r[:, 128 * t : 128 * (t + 1)],
                        rhs=w2gh_sb[:, :],
                        start=True, stop=False,
                    )
                    nc.tensor.matmul(
                        pZg[:, C2 * t : C2 * (t + 1)],
                        lhsT=sH1r[:, 128 * t : 128 * (t + 1)],
                        rhs=w2gl_sb[:, :],
                        start=False, stop=True,
                        skip_group_check=True,
                    )
                sZg = wpool.tile([128, 4 * C2], F32R, tag="sZg")
                nc.vector.tensor_copy(sZg[:, :], pZg[:, :])

                # ---- drain previous wave (its sZg has long settled) ----
                if prev is not None:
                    drain_mp2(prev)
                    drain(prev)

                prev = {"pZZ": pZZ, "sZg": sZg, "bd_v": bd_v, "w": w,
                        "sG_blk": sG_blk, "b0": b0}

            drain_mp2(prev)
            drain(prev)

            nc.sync.dma_start(
                out=y_d.rearrange("(o b) one -> o (b one)", o=1), in_=y_acc[:, :]
            )

    _split_multi_waits(nc)
    return nc


def _prep_host(x, a, weights):
    """Host-side layout/precision prep (pure repacking, no graph math)."""
    at = np.ascontiguousarray(a.transpose(0, 2, 1))
    xh = x.astype(ml_dtypes.bfloat16).astype(np.float32)
    xl = (x - xh).astype(np.float32)
    x2 = np.empty(x.shape[:-1] + (2 * FIN,), np.float32)
    x2[..., 0::2] = xh
    x2[..., 1::2] = xl
    w1 = weights["w1"]
    w1h = w1.astype(ml_dtypes.bfloat16).astype(np.float32)
    w1l = (w1 - w1h).astype(np.float32)
    w2g = weights["w2g"]
    w2gh = w2g.astype(np.float16)
    w2gl = (w2g - w2gh.astype(np.float32)).astype(np.float16)
    return {
        "at": at, "x2": x2,
        "w1h2": np.repeat(w1h, 2, axis=0),
        "w1l2": np.repeat(w1l, 2, axis=0),
        "w2gh": w2gh, "w2gl": w2gl,
        "w2s": weights["w2s"].astype(np.float16),
        "b1": weights["b1"], "b2": weights["b2"],
        "wf1": weights["wf1"], "bf1": weights["bf1"],
        "wf2": weights["wf2"], "bf2": weights["bf2"],
    }


def kernel(**inputs) -> np.ndarray:
    x = np.asarray(inputs["x"], dtype=np.float32)
    a = np.asarray(inputs["a"], dtype=np.float32)
    weights = {
        k: np.asarray(inputs[k], dtype=np.float32)
        for k in ("w1", "b1", "w2g", "w2s", "b2", "wf1", "bf1", "wf2", "bf2")
    }
    full = _prep_host(x, a, weights)

    nc = build_nc(B_CORE)
    in_maps = []
    for c in range(N_CORES):
        sl = slice(c * B_CORE, (c + 1) * B_CORE)
        m = {k: (v[sl] if k in ("at", "x2") else v) for k, v in full.items()}
        in_maps.append(m)

    res = run_bass_kernel_spmd(nc, in_maps, list(range(N_CORES)))
    outs = [res.results[c]["y"] for c in range(N_CORES)]
    return np.concatenate(outs, axis=0).astype(np.float32)


if __name__ == "__main__":
    # quick 1-core correctness check against numpy on a small slice
    rng = np.random.default_rng(0)
    bc = 512
    x = rng.standard_normal((bc, NN, FIN), dtype=np.float32)
    a = rng.random((bc, NN, NN), dtype=np.float32)
    w = {
        "w1": rng.standard_normal((FIN, C1), dtype=np.float32) * 0.2,
        "b1": rng.standard_normal(C1).astype(np.float32) * 0.1,
        "w2g": rng.standard_normal((C1, C2), dtype=np.float32) * 0.1,
        "w2s": rng.standard_normal((C1, C2), dtype=np.float32) * 0.1,
        "b2": rng.standard_normal(C2).astype(np.float32) * 0.1,
        "wf1": rng.standard_normal((C2, C3), dtype=np.float32) * 0.1,
        "bf1": rng.standard_normal(C3).astype(np.float32) * 0.1,
        "wf2": rng.standard_normal((C3, 1), dtype=np.float32) * 0.3,
        "bf2": rng.standard_normal(1).astype(np.float32) * 0.1,
    }
    full = _prep_host(x, a, w)
    nc = build_nc(bc)
    res = run_bass_kernel_spmd(nc, [full], [0])
    y = res.results[0]["y"][:, 0]

    AX = a @ x
    H1 = np.maximum(AX @ w["w1"] + w["b1"], 0)
    H2 = np.maximum(a @ (H1 @ w["w2g"]) + H1 @ w["w2s"] + w["b2"], 0)
    g = H2.sum(axis=1)
    g1 = np.maximum(g @ w["wf1"] + w["bf1"], 0)
    z = g1 @ w["wf2"] + w["bf2"]
    yref = (1 / (1 + np.exp(-z)))[:, 0]
    print("max abs err:", np.abs(y - yref).max(), " ref max:", np.abs(yref).max())


# revision 5
# speedup vs baseline: 1.0248x; 1.0003x over previous
"""Trainium2 Bass kernel v4 for batched tiny-graph GNN (B=32768, N=22).

Numerics: fp32r matmuls (~12-bit mantissa, bf16-class speed) with exactness
restored where it matters:
  * X shipped as interleaved bf16-exact split (xh, xl); MP1 contracts the
    pair in ONE matmul via K-stacking (lhsT [128, 32]).
  * W1 shipped row-duplicated (w1h2/w1l2 [32, 128]) to consume the
    interleaved AX pair directly; 2 matmuls restore W1 exactly.
  * W2g shipped column-interleaved (w2g2 [128, 128]); hi/lo halves summed by
    one strided DVE add after the Zg matmul.
  * W2s single fp32r; final MLP true fp32 batched per block; A fp32r
    (host-transposed, no PE transposes).

Per wave (16 graphs): MP1 4x88c, d1 2x352c, Zs 1x352c, Zg 4x128c, MP2 4x88c.
Block-diagonal A^T built once per 128-graph block by 4 large scatter copies.
"""

import sys

sys.path.insert(0, "/opt/trn_rl_repo")

import numpy as np
import ml_dtypes

import concourse.bass as bass
import concourse.mybir as mybir
import concourse.tile as tile
from concourse.bass_utils import run_bass_kernel_spmd

import bass_rust


def _patched_drain_and_barrier(self, tick_clock, wait_clock):
    """Walrus in this container cannot encode multi-wait Drain instructions;
    spread the tile-exit sem waits across single-wait NOPs instead."""
    from concourse.tile import ScopedClock

    probe = self.nc.sync.nop(hint="drain_wait_split")
    wait_clock.add_sem_waits(probe.ins, ScopedClock({None: tick_clock.global_clock}))
    si = probe.ins.sync_info
    waits = list(si.on_wait) if si is not None else []
    probe.ins.sync_info = bass_rust.SyncInfo(on_wait=waits[:1], on_update=[])
    for w in waits[1:]:
        n = self.nc.sync.nop(hint="drain_wait_split")
        n.ins.sync_info = bass_rust.SyncInfo(on_wait=[w], on_update=[])
    self.nc.sync.drain()
    self.nc.all_engine_barrier()
    assert self.sems is not None
    popped = self.nc._tile_sem_poison_stack.pop()
    assert popped is self._sem_poison
    self.nc.clear_and_free_semaphores(list(self.sems.allocated().values()))
    self.nc.all_engine_barrier()


tile.TileContext._drain_and_barrier = _patched_drain_and_barrier

F32 = mybir.dt.float32
F32R = mybir.dt.float32r
F16 = mybir.dt.float16

B_TOTAL = 32768
N_CORES = 8
B_CORE = B_TOTAL // N_CORES          # 4096
NN = 22
FIN = 16
C1, C2, C3 = 128, 64, 32
WAVE = 16                            # graphs per wave (4 tetrads)
BLK = 256                            # graphs per DMA ingest block
AFT = mybir.ActivationFunctionType

_split_ctr = [0]


def _split_multi_waits(nc):
    """This container's walrus encodes at most one sem wait per instruction:
    hoist extra waits onto same-engine NOPs inserted just before."""
    for f in nc.m.functions:
        for bb in f.blocks:
            out = []
            for inst in bb.instructions:
                si = inst.sync_info
                if si is not None and len(si.on_wait) > 1:
                    waits = list(si.on_wait)
                    for w in waits[:-1]:
                        _split_ctr[0] += 1
                        n = mybir.InstNoOp(
                            name=f"waitsplit_{_split_ctr[0]}", ins=[], outs=[]
                        )
                        n.engine = inst.engine
                        n.sync_info = bass_rust.SyncInfo(on_wait=[w], on_update=[])
                        out.append(n)
                        nc.register_instruction(n, overwrite=True)
                    inst.sync_info = bass_rust.SyncInfo(
                        on_wait=[waits[-1]], on_update=list(si.on_update)
                    )
                out.append(inst)
            bb.instructions = out


def build_nc(b_core: int = B_CORE, blk: int = BLK) -> bass.Bass:
    assert b_core % blk == 0
    n_blocks = b_core // blk
    waves_per_blk = blk // WAVE

    nc = bass.Bass()

    at_d = nc.declare_dram_parameter("at", [b_core, NN, NN], F32R, isOutput=False)
    x2_d = nc.declare_dram_parameter("x2", [b_core, NN, 2 * FIN], F32R, isOutput=False)
    w1h_d = nc.declare_dram_parameter("w1h2", [2 * FIN, C1], F32R, isOutput=False)
    w1l_d = nc.declare_dram_parameter("w1l2", [2 * FIN, C1], F32R, isOutput=False)
    w2gh_d = nc.declare_dram_parameter("w2gh", [C1, C2], F16, isOutput=False)
    w2gl_d = nc.declare_dram_parameter("w2gl", [C1, C2], F16, isOutput=False)
    w2s_d = nc.declare_dram_parameter("w2s", [C1, C2], F16, isOutput=False)
    b1_d = nc.declare_dram_parameter("b1", [C1], F32, isOutput=False)
    b2_d = nc.declare_dram_parameter("b2", [C2], F32, isOutput=False)
    wf1_d = nc.declare_dram_parameter("wf1", [C2, C3], F32, isOutput=False)
    bf1_d = nc.declare_dram_parameter("bf1", [C3], F32, isOutput=False)
    wf2_d = nc.declare_dram_parameter("wf2", [C3, 1], F32, isOutput=False)
    bf2_d = nc.declare_dram_parameter("bf2", [1], F32, isOutput=False)
    y_d = nc.declare_dram_parameter("y", [b_core, 1], F32, isOutput=True)

    CMP = 4 * NN          # 88 compact cols per tetrad
    CW = 4 * CMP          # 352 compact cols per wave

    with tile.TileContext(nc) as tc:
        with (
            tc.tile_pool(name="const", bufs=1) as cpool,
            tc.tile_pool(name="ioa", bufs=2) as iopool,
            tc.tile_pool(name="iox", bufs=2) as ioxpool,
            tc.tile_pool(name="bdp", bufs=2) as bdpool,
            tc.tile_pool(name="gio", bufs=2) as gpool,
            tc.tile_pool(name="work", bufs=2) as wpool,
            tc.tile_pool(name="pA", bufs=2, space="PSUM") as pApool,
            tc.tile_pool(name="pH", bufs=2, space="PSUM") as pHpool,
            tc.tile_pool(name="pZ", bufs=2, space="PSUM") as pZpool,
            tc.tile_pool(name="pZg", bufs=1, space="PSUM") as pZgpool,
        ):
            # ---- persistent constants (all f32; bitcast to f32r at use) ----
            def wload(dram, p, q, tag, dt=F32):
                t = cpool.tile([p, q], dt, tag=tag)
                nc.sync.dma_start(out=t[:, :], in_=dram[:, :])
                return t

            w1h_sb = wload(w1h_d, 2 * FIN, C1, "w1h2", F32R)
            w1l_sb = wload(w1l_d, 2 * FIN, C1, "w1l2", F32R)
            w2gh_sb = wload(w2gh_d, C1, C2, "w2gh", F16)
            w2gl_sb = wload(w2gl_d, C1, C2, "w2gl", F16)
            w2s_sb = wload(w2s_d, C1, C2, "w2s", F16)
            wf1_sb = wload(wf1_d, C2, C3, "wf1")
            wf2t = cpool.tile([C3, 1], F32, tag="wf2")
            nc.sync.dma_start(out=wf2t[:, :], in_=wf2_d[:, :])

            def bload(dram, p, tag):
                t = cpool.tile([p, 1], F32, tag=tag)
                nc.sync.dma_start(out=t[:, :], in_=dram.rearrange("(c o) -> c o", o=1))
                return t

            b1c = bload(b1_d, C1, "b1")
            b2c = bload(b2_d, C2, "b2")
            bf1c = bload(bf1_d, C3, "bf1")
            bf2c = bload(bf2_d, 1, "bf2")
            y_acc = cpool.tile([1, b_core], F32, tag="yacc")

            # padded H1^T tiles (gap cols harmless: bd gap rows are zero)
            h1s = []
            for par in range(2):
                h1 = cpool.tile([C1, 4 * 128], F16, tag=f"h1p{par}")
                nc.vector.memset(h1[:, :], 0.0)
                h1s.append(h1)

            total_waves = n_blocks * waves_per_blk
            prev = None
            x2v = bd_v = None
            sG_blk = None
            b0 = 0

            def drain_mp2(pv):
                for t in range(4):
                    nc.tensor.matmul(
                        pv["pZZ"][:, CMP * t : CMP * (t + 1)],
                        lhsT=pv["sZg"][:, C2 * t : C2 * (t + 1)],
                        rhs=pv["bd_v"][:, 4 * pv["w"] + t, :, 0:NN],
                        start=False, stop=(t == 3),
                        skip_group_check=True,
                    )

            def drain(pv):
                # previous wave: H2 + pool (+ block finals)
                sH2T = wpool.tile([C2, CW], F32, tag="sH2T")
                nc.scalar.activation(
                    out=sH2T[:, :], in_=pv["pZZ"][:, :], func=AFT.Relu,
                    bias=b2c[:, :],
                )
                nc.vector.reduce_sum(
                    out=pv["sG_blk"][:, WAVE * pv["w"] : WAVE * (pv["w"] + 1)]
                    .rearrange("p (t g) -> p t g", t=4),
                    in_=sH2T.rearrange("p (t g q) -> p t g q", t=4, g=4),
                    axis=mybir.AxisListType.X,
                )
                if pv["w"] == waves_per_blk - 1:
                    pGb = pZgpool.tile([128, 4 * C2], F32, tag="pZg")
                    pG1 = pGb[0:C3, 0:blk]
                    nc.tensor.matmul(pG1, lhsT=wf1_sb[:, :], rhs=pv["sG_blk"][:, :])
                    sG1 = gpool.tile([C3, blk], F32, tag="sG1")
                    nc.scalar.activation(
                        out=sG1[:, :], in_=pG1, func=AFT.Relu, bias=bf1c[:, :]
                    )
                    pY = pGb[32:33, 0:blk]
                    nc.tensor.matmul(pY, lhsT=wf2t[:, :], rhs=sG1[:, :])
                    nc.scalar.activation(
                        out=y_acc[:, pv["b0"] : pv["b0"] + blk], in_=pY,
                        func=AFT.Sigmoid, bias=bf2c[:, :],
                    )

            prim = cpool.tile([128, 128], F16, tag="prim")
            nc.vector.memset(prim[:, :], 0.0)
            scratch = nc.alloc_psum_tensor("ham_scratch", [128, 128], F32)

            def filler(n):
                for _ in range(n):
                    nc.tensor.matmul(scratch[:, :], lhsT=prim[:, :],
                                     rhs=prim[:, :], start=True, stop=True,
                                     skip_group_check=True)

            filler(72)

            def ingest(kblk):
                # issue the DMAs + allocate tiles for block kblk
                kb0 = kblk * blk
                at_blk = iopool.tile([128, (blk // 4) * NN], F32R, tag="at_blk")
                at_src = at_d[kb0 : kb0 + blk].rearrange(
                    "(s g) m n -> g m s n", g=4
                )
                for g in range(4):
                    nc.sync.dma_start(
                        out=at_blk[32 * g : 32 * g + NN].rearrange(
                            "p (s n) -> p s n", n=NN
                        ),
                        in_=at_src[g],
                    )
                x2_blk = ioxpool.tile(
                    [128, (blk // 4) * 2 * FIN], F32R, tag="x2_blk"
                )
                x2_src = x2_d[kb0 : kb0 + blk].rearrange(
                    "(s g) m c -> g m s c", g=4
                )
                for g in range(4):
                    nc.sync.dma_start(
                        out=x2_blk[32 * g : 32 * g + NN].rearrange(
                            "p (s c) -> p s c", c=2 * FIN
                        ),
                        in_=x2_src[g],
                    )
                bd_all = bdpool.tile([128, (blk // 4) * 128], F32R, tag="bd_all")
                if kblk < 2:
                    nc.vector.memset(bd_all[:, :].bitcast(F32), 0.0)
                return {"at_blk": at_blk, "x2_blk": x2_blk, "bd_all": bd_all}

            def scatter_slice(blkh, j):
                # one slice (1/waves_per_blk) of the BD scatter, on gpsimd
                g, half = j // (waves_per_blk // 4), j % (waves_per_blk // 4)
                sw = (blk // 4) // (waves_per_blk // 4)
                at_v = blkh["at_blk"][:, :].rearrange("p (s n) -> p s n", n=NN)
                dst = blkh["bd_all"][32 * g : 32 * g + NN].rearrange(
                    "p (s c) -> p s c", c=128
                )[:, half * sw : (half + 1) * sw, 32 * g : 32 * g + NN]
                srcv = at_v[32 * g : 32 * g + NN, half * sw : (half + 1) * sw]
                nc.gpsimd.tensor_copy(dst, srcv)

            cur = ingest(0)
            for j in range(waves_per_blk):
                scatter_slice(cur, j)
            nxt = ingest(1) if n_blocks > 1 else None

            for wv in range(total_waves):
                iblk, w = wv // waves_per_blk, wv % waves_per_blk
                if w == 0:
                    b0 = iblk * blk
                    x2v = cur["x2_blk"][:, :].rearrange(
                        "p (s c) -> p s c", c=2 * FIN
                    )
                    bd_v = cur["bd_all"][:, :].rearrange(
                        "p (s g q) -> p s g q", g=4, q=32
                    )
                    sG_blk = gpool.tile([C2, blk], F32, tag="sG_blk")
                    if iblk > 0:
                        filler(28)
                # prefetch next block: scatter one slice per wave on gpsimd
                if nxt is not None:
                    scatter_slice(nxt, w)
                    if w == waves_per_blk - 1:
                        cur = nxt
                        nxt = ingest(iblk + 2) if iblk + 2 < n_blocks else None

                sH1T = h1s[wv % 2]

                def bd_c(t):
                    return bd_v[:, 4 * w + t, :, 0:NN]

                # ---- MP1: one K-stacked matmul per tetrad ----
                pAXT = pApool.tile([2 * FIN, CW], F32, tag="pAXT")
                for t in range(4):
                    s = 4 * w + t
                    nc.tensor.matmul(
                        pAXT[:, CMP * t : CMP * (t + 1)],
                        lhsT=x2v[:, s, :],
                        rhs=bd_c(t),
                        start=True, stop=True,
                    )
                sAXT = wpool.tile([2 * FIN, CW], F32R, tag="sAXT")
                nc.vector.tensor_copy(sAXT[:, :], pAXT[:, :])

                # ---- dense1 (w1 pair, K-stacked): H1^T = relu(. + b1) ----
                pH1 = pHpool.tile([C1, CW], F32, tag="pH1")
                nc.tensor.matmul(pH1[:, :], lhsT=w1h_sb[:, :], rhs=sAXT[:, :],
                                 start=True, stop=False)
                nc.tensor.matmul(pH1[:, :], lhsT=w1l_sb[:, :], rhs=sAXT[:, :],
                                 start=False, stop=True,
                                 skip_group_check=True)
                h1_strided = sH1T.rearrange("p (t g q) -> p t g q", t=4, g=4)[
                    :, :, :, 0:NN
                ]
                nc.scalar.activation(
                    out=h1_strided, in_=pH1[:, :], func=AFT.Relu, bias=b1c[:, :]
                )
                sH1r = sH1T[:, :]
                h1_rhs = sH1r.rearrange("p (t g q) -> p t g q", t=4, g=4)[
                    :, :, :, 0:NN
                ]

                # ---- Zs into pZZ; Zg pair (fp16) ----
                pZZ = pZpool.tile([C2, CW], F32, tag="pZZ")
                nc.tensor.matmul(pZZ[:, :], lhsT=w2s_sb[:, :],
                                 rhs=h1_rhs, start=True, stop=False)
                pZg = pZgpool.tile([128, 4 * C2], F32, tag="pZg")
                for t in range(4):
                    nc.tensor.matmul(
                        pZg[:, C2 * t : C2 * (t + 1)],
                        lhsT=sH1r[:, 128 * t : 128 * (t + 1)],
                        rhs=w2gh_sb[:, :],
                        start=True, stop=False,
                    )
                    nc.tensor.matmul(
                        pZg[:, C2 * t : C2 * (t + 1)],
                        lhsT=sH1r[:, 128 * t : 128 * (t + 1)],
                        rhs=w2gl_sb[:, :],
                        start=False, stop=True,
                        skip_group_check=True,
                    )
                sZg = wpool.tile([128, 4 * C2], F32R, tag="sZg")
                nc.vector.tensor_copy(sZg[:, :], pZg[:, :])

                # ---- drain previous wave (its sZg has long settled) ----
                if prev is not None:
                    drain_mp2(prev)
                    drain(prev)

                prev = {"pZZ": pZZ, "sZg": sZg, "bd_v": bd_v, "w": w,
                        "sG_blk": sG_blk, "b0": b0}

            drain_mp2(prev)
            drain(prev)

            nc.sync.dma_start(
                out=y_d.rearrange("(o b) one -> o (b one)", o=1), in_=y_acc[:, :]
            )

    _split_multi_waits(nc)
    return nc


def _prep_host(x, a, weights):
    """Host-side layout/precision prep (pure repacking, no graph math)."""
    at = np.ascontiguousarray(a.transpose(0, 2, 1))
    xh = x.astype(ml_dtypes.bfloat16).astype(np.float32)
    xl = (x - xh).astype(np.float32)
    x2 = np.empty(x.shape[:-1] + (2 * FIN,), np.float32)
    x2[..., 0::2] = xh
    x2[..., 1::2] = xl
    w1 = weights["w1"]
    w1h = w1.astype(ml_dtypes.bfloat16).astype(np.float32)
    w1l = (w1 - w1h).astype(np.float32)
    w2g = weights["w2g"]
    w2gh = w2g.astype(np.float16)
    w2gl = (w2g - w2gh.astype(np.float32)).astype(np.float16)
    return {
        "at": at, "x2": x2,
        "w1h2": np.repeat(w1h, 2, axis=0),
        "w1l2": np.repeat(w1l, 2, axis=0),
        "w2gh": w2gh, "w2gl": w2gl,
        "w2s": weights["w2s"].astype(np.float16),
        "b1": weights["b1"], "b2": weights["b2"],
        "wf1": weights["wf1"], "bf1": weights["bf1"],
        "wf2": weights["wf2"], "bf2": weights["bf2"],
    }


def kernel(**inputs) -> np.ndarray:
    x = np.asarray(inputs["x"], dtype=np.float32)
    a = np.asarray(inputs["a"], dtype=np.float32)
    weights = {
        k: np.asarray(inputs[k], dtype=np.float32)
        for k in ("w1", "b1", "w2g", "w2s", "b2", "wf1", "bf1", "wf2", "bf2")
    }
    full = _prep_host(x, a, weights)

    nc = build_nc(B_CORE)
    in_maps = []
    for c in range(N_CORES):
        sl = slice(c * B_CORE, (c + 1) * B_CORE)
        m = {k: (v[sl] if k in ("at", "x2") else v) for k, v in full.items()}
        in_maps.append(m)

    res = run_bass_kernel_spmd(nc, in_maps, list(range(N_CORES)))
    outs = [res.results[c]["y"] for c in range(N_CORES)]
    return np.concatenate(outs, axis=0).astype(np.float32)


if __name__ == "__main__":
    # quick 1-core correctness check against numpy on a small slice
    rng = np.random.default_rng(0)
    bc = 512
    x = rng.standard_normal((bc, NN, FIN), dtype=np.float32)
    a = rng.random((bc, NN, NN), dtype=np.float32)
    w = {
        "w1": rng.standard_normal((FIN, C1), dtype=np.float32) * 0.2,
        "b1": rng.standard_normal(C1).astype(np.float32) * 0.1,
        "w2g": rng.standard_normal((C1, C2), dtype=np.float32) * 0.1,
        "w2s": rng.standard_normal((C1, C2), dtype=np.float32) * 0.1,
        "b2": rng.standard_normal(C2).astype(np.float32) * 0.1,
        "wf1": rng.standard_normal((C2, C3), dtype=np.float32) * 0.1,
        "bf1": rng.standard_normal(C3).astype(np.float32) * 0.1,
        "wf2": rng.standard_normal((C3, 1), dtype=np.float32) * 0.3,
        "bf2": rng.standard_normal(1).astype(np.float32) * 0.1,
    }
    full = _prep_host(x, a, w)
    nc = build_nc(bc)
    res = run_bass_kernel_spmd(nc, [full], [0])
    y = res.results[0]["y"][:, 0]

    AX = a @ x
    H1 = np.maximum(AX @ w["w1"] + w["b1"], 0)
    H2 = np.maximum(a @ (H1 @ w["w2g"]) + H1 @ w["w2s"] + w["b2"], 0)
    g = H2.sum(axis=1)
    g1 = np.maximum(g @ w["wf1"] + w["bf1"], 0)
    z = g1 @ w["wf2"] + w["bf2"]
    yref = (1 / (1 + np.exp(-z)))[:, 0]
    print("max abs err:", np.abs(y - yref).max(), " ref max:", np.abs(yref).max())
